# revision 5
# baseline (speedup 1.0000x reference)
"""Trainium2 Bass kernel v2 for nn_LowRankLayer_dilation (B=4, C=64, H=W=128).

Math (exact collapse of the rank-3 NMF update, eps negligible):
    h   = relu(W_head @ x)            (per-pixel channel matmul)
    g   = W_tail @ h                  (per-pixel channel matmul)
    a   = box9(h)                     (3x3 dilation-2 box sum, edge-clamped)
    n_k = sum_c (a/9)_c * h_c(p+d_k)  (9 taps, d in {-2,0,2}^2)
    out = x + (n_4 / sum_j n_j^2) * sum_k n_k * g(p+d_k)

v2 changes vs baseline:
  - bf16 everywhere incl. residual + output (no f32 xr input, y is bf16,
    host upcasts): input DMA 3.4MB -> 1.2MB, output 2.1MB -> 1MB.
  - input streamed in 512-col chunks on the sync engine so head matmuls
    start as soon as the first chunk lands.
  - engine rebalance: scalar owns all PSUM->SBUF copies + relu + squares,
    vector owns 2-input elementwise + reciprocal, gpsimd owns nst/y DMA
    triggers + half the residual chain, sync owns input DMA triggers.
  - box filter split: av for half 0 computed first so the k-loop starts
    ~8us earlier; av for half 1 filled in during half 0's k-loop.
  - nst rows collected with one partition-strided DMA per (k, half).

Sharding: 8 cores = (batch b, H-half). Each core gets a 68-row halo'd
slice packed as 2 channel blocks on 128 partitions: partition p = c +
64*blk, blk A = slice rows 0..35, blk B = rows 32..67. h and g stored
with 2 replicate-padded columns per side (row stride 132) so every
dilated tap is a strided AP view.
"""
import sys
import contextlib
import numpy as np

sys.path.insert(0, '/opt/trn_rl_repo')

import concourse.bass as bass  # noqa: E402,F401
import concourse.bacc as bacc  # noqa: E402
import concourse.tile as tile  # noqa: E402
import concourse.mybir as mybir  # noqa: E402
from concourse.bass_utils import run_bass_kernel_spmd  # noqa: E402

F32 = mybir.dt.float32
BF16 = mybir.dt.bfloat16
AT = mybir.ActivationFunctionType
OP = mybir.AluOpType

N_CORES = 8
RIN = 36          # per-block input rows (with +-2 halo)
ROUT = 32         # per-block output rows
W = 128
WP = W + 4        # padded row stride for h/g
FIN = RIN * W     # 4608
FOUT = ROUT * W   # 4096
OFFS = [(di, dj) for di in (-2, 0, 2) for dj in (-2, 0, 2)]
EDT = BF16


def _build():
    nc = bacc.Bacc("TRN2", target_bir_lowering=False, debug=False,
                   num_devices=N_CORES)
    xb_ext = nc.dram_tensor("xb", [128, FIN], EDT, kind="ExternalInput").ap()
    w2_ext = nc.dram_tensor("w2", [128, 128], EDT, kind="ExternalInput").ap()
    w3_ext = nc.dram_tensor("w3", [128, 128], EDT, kind="ExternalInput").ap()
    bo_ext = nc.dram_tensor("bo", [128, 128], EDT, kind="ExternalInput").ap()
    s4a_ext = nc.dram_tensor("s4a", [18, 128], EDT, kind="ExternalInput").ap()
    s4b_ext = nc.dram_tensor("s4b", [18, 128], EDT, kind="ExternalInput").ap()
    bc2_ext = nc.dram_tensor("bc2", [2, 128], EDT, kind="ExternalInput").ap()
    id_ext = nc.dram_tensor("idm", [128, 128], EDT, kind="ExternalInput").ap()
    y_ext = nc.dram_tensor("y", [128, FOUT], EDT, kind="ExternalOutput").ap()

    with tile.TileContext(nc) as tc, contextlib.ExitStack() as ctx:
        cpool = ctx.enter_context(tc.tile_pool(name="consts", bufs=1))
        big = ctx.enter_context(tc.tile_pool(name="big", bufs=1))
        ppool = ctx.enter_context(tc.tile_pool(name="prod", bufs=4))
        npool = ctx.enter_context(tc.tile_pool(name="nbuf", bufs=4))

        # weights first so the head matmuls can start on chunk 0
        # ascending chunk sizes: the first matmul is gated only on a small
        # 256-col chunk instead of a quarter of the input
        # spray the input triggers across otherwise-idle engines so the
        # descriptor generation runs in parallel and the head's first
        # chunks land sooner
        w2 = cpool.tile([128, 128], EDT)
        nc.sync.dma_start(w2[:], w2_ext[:])
        xbt = big.tile([128, FIN], EDT)
        bounds = [0, 512, 1536, 2560, 3584, FIN]
        engs = [nc.scalar, nc.gpsimd, nc.sync, nc.scalar, nc.gpsimd]
        for c in range(5):
            engs[c].dma_start(xbt[:, bounds[c]:bounds[c + 1]],
                              xb_ext[:, bounds[c]:bounds[c + 1]])
        w3 = cpool.tile([128, 128], EDT)
        nc.scalar.dma_start(w3[:], w3_ext[:])
        bo = cpool.tile([128, 128], EDT)
        nc.sync.dma_start(bo[:], bo_ext[:])
        s4a = cpool.tile([18, 128], EDT)
        nc.gpsimd.dma_start(s4a[:], s4a_ext[:])
        s4b = cpool.tile([18, 128], EDT)
        nc.gpsimd.dma_start(s4b[:], s4b_ext[:])
        bc2 = cpool.tile([2, 128], EDT)
        nc.gpsimd.dma_start(bc2[:], bc2_ext[:])
        idm = cpool.tile([128, 128], EDT)
        nc.gpsimd.dma_start(idm[:], id_ext[:])
        warm = cpool.tile([1, 2], EDT)
        nc.scalar.copy(warm[:], w2[0:1, 0:2])

        # h/g: (RIN, WP) rows; data at cols 2..129, replicate pads at 0,1,
        # 130,131. A (di,dj) tap over out rows = strided view.
        hf = big.tile([128, RIN * WP], EDT)
        h3 = hf.rearrange("p (r w) -> p r w", w=WP)
        xbt3 = xbt.rearrange("p (r w) -> p r w", w=W)

        def tap(t3, di, dj, rows=ROUT, r0=2):
            rr = r0 + di
            return t3[:, rr:rr + rows, 2 + dj:2 + dj + W]

        # ---- head + tail matmuls: h = relu(W_head @ x), g = W_tail @ h ----
        T = big.tile([128, FIN], EDT)
        T3 = T.rearrange("p (r w) -> p r w", w=W)
        av = big.tile([128, FOUT], EDT)
        av3 = av.rearrange("p (r w) -> p r w", w=W)

        def pads(t3, lo, hi):
            for dst, src in ((0, 2), (1, 2), (130, 129), (131, 129)):
                nc.vector.tensor_copy(t3[:, lo:hi, dst:dst + 1],
                                      t3[:, lo:hi, src:src + 1])

        def box_part(lo, hi):
            # T rows lo..hi (column 3-tap sums; T row r needs only h row r)
            nc.vector.tensor_add(T3[:, lo:hi], tap(h3, -2, -2, hi - lo, 2 + lo),
                                 tap(h3, -2, 0, hi - lo, 2 + lo))
            nc.vector.tensor_add(T3[:, lo:hi], T3[:, lo:hi],
                                 tap(h3, -2, 2, hi - lo, 2 + lo))

        # head/tail in 3 row-groups of 12 (1536 cols, 3 PSUM banks each,
        # ring of 2 = 6 banks). Box pieces follow the relus row-wise so
        # the half-0 k loop can start as soon as h rows 0..19 exist.
        with tc.tile_pool(name="psmm", bufs=2, space="PSUM") as psmm:
            G = 1536

            def head_mm(j):
                ps = psmm.tile([128, G], F32)
                for q in range(3):
                    c0 = j * G + q * 512
                    nc.tensor.matmul(ps[:, q * 512:(q + 1) * 512], w2[:],
                                     xbt[:, c0:c0 + 512],
                                     start=True, stop=True)
                return ps

            hps0 = head_mm(0)
            hps1 = head_mm(1)
            nc.vector.tensor_relu(h3[:, 0:12, 2:2 + W],
                                  hps0.rearrange("p (r w) -> p r w", w=W))
            hps2 = head_mm(2)                 # ring slot 0, after relu j0
            nc.scalar.activation(h3[:, 12:24, 2:2 + W],
                                 hps1.rearrange("p (r w) -> p r w", w=W),
                                 AT.Relu)
            nc.scalar.activation(h3[:, 24:36, 2:2 + W],
                                 hps2.rearrange("p (r w) -> p r w", w=W),
                                 AT.Relu)
            for dst, src in ((0, 2), (1, 2), (130, 129), (131, 129)):
                nc.scalar.copy(h3[:, 20:36, dst:dst + 1],
                               h3[:, 20:36, src:src + 1])
            pads(h3, 0, 12)
            box_part(0, 12)
            pads(h3, 12, 20)
            box_part(12, 20)
            nc.vector.tensor_add(av3[:, 0:16], T3[:, 0:16], T3[:, 2:18])
            nc.vector.tensor_add(av3[:, 0:16], av3[:, 0:16], T3[:, 4:20])

        # ---- per-k: n_k (PE reduce+broadcast), F accumulated on the PE ----
        # PSUM budget (8 banks): psnk tag ring 3x[128,1024]f32 = 6 banks
        # (shared by bo outputs, the deferred q1 accumulator, and the cf
        # chain's s2/bc2 outputs), psfa 1x[128,1024] = 2 banks (q0
        # accumulator). F accumulation is split in column quarters: q0
        # accumulates in-loop (lag 2); q1's 18 matmuls run as one dense
        # deferred stream that keeps the PE busy across the half boundary.
        nst = cpool.tile([66, FOUT], EDT)       # n_k rows, row pair by kr
        # rows 64:66 hold a second copy of the center rows so the q1 cf
        # mul can read both operands at base partition 64 (verifier
        # requires equal SB base partitions)
        fa = big.tile([128, FOUT], EDT)         # facc copied out per quarter
        nsq = npool.tile([18, FOUT], EDT, tag="nsq", bufs=1)
        cfr = cpool.tile([2, FOUT], EDT)
        HF = 2048
        QF = 1024

        with tc.tile_pool(name="psnk", bufs=3, space="PSUM") as psnk, \
                tc.tile_pool(name="psfa", bufs=1, space="PSUM") as psfa:

            def cf_steps(half):
                """Deferred tail for one half, pipelined per column
                quarter: nsq square (scalar) -> s2 matmuls -> reciprocal +
                cf mul (vector) -> bc2 broadcast -> cfb copy (scalar) ->
                residual (vector) -> DMA out (sync)."""
                rh = half * 16
                s2ps = psnk.tile([128, QF], F32, tag="nk")
                for qq in range(2):
                    q0 = half * HF + qq * QF
                    nc.scalar.activation(nsq[:, q0:q0 + QF],
                                         nst[0:18, q0:q0 + QF], AT.Square)
                    sbw = s4a if qq == 0 else s4b
                    for q in range(2):
                        c0 = q0 + q * 512
                        nc.tensor.matmul(s2ps[:, q * 512:(q + 1) * 512],
                                         sbw[:], nsq[:, c0:c0 + 512],
                                         start=(qq == 0), stop=(qq == 1))
                    yield
                rcp = npool.tile([66, QF], F32, tag="rcp", bufs=2)
                nc.vector.reciprocal_approx_fast(rcp[:], s2ps[0:66, :])
                for qq in range(2):
                    q0 = half * HF + qq * QF
                    b = 64 * qq
                    nc.vector.tensor_mul(cfr[:, q0:q0 + QF],
                                         nst[b:b + 2, q0:q0 + QF],
                                         rcp[b:b + 2, :])
                    yield
                for qq in range(2):
                    q0 = half * HF + qq * QF
                    cfbs = npool.tile([128, QF], EDT, tag="cfbs", bufs=2)
                    pst = psnk.tile([128, QF], F32, tag="nk")
                    for q in range(2):
                        c0 = q0 + q * 512
                        nc.tensor.matmul(pst[:, q * 512:(q + 1) * 512],
                                         bc2[:], cfr[:, c0:c0 + 512],
                                         start=True, stop=True)
                    nc.scalar.copy(cfbs[:], pst[:])
                    yield
                    res = npool.tile([128, QF], EDT, tag="res", bufs=2)
                    nc.vector.tensor_mul(res[:], fa[:, q0:q0 + QF], cfbs[:])
                    r3 = res.rearrange("p (r w) -> p r w", w=W)
                    nc.vector.tensor_add(
                        r3[:], r3[:],
                        xbt3[:, 2 + rh + qq * 8:10 + rh + qq * 8, :])
                    nc.sync.dma_start(y_ext[:, q0:q0 + QF], res[:])
                    yield

            # half-1 box filter on the PE: av[16:32] accumulated as 9
            # identity matmuls over the dilated taps. Started at build
            # top so the first burst fills the PE's startup gap (gated
            # only on relu j2 + the scalar pads above).
            def make_avb():
                for qq in range(2):
                    acc = psnk.tile([128, QF], F32, tag="nk")
                    for j, (di2, dj2) in enumerate(OFFS):
                        for q in range(2):
                            tv = tap(h3, di2, dj2, rows=4,
                                     r0=18 + 8 * qq + 4 * q)
                            nc.tensor.matmul(
                                acc[:, q * 512:(q + 1) * 512], idm[:],
                                tv, start=(j == 0), stop=(j == 8))
                        if j in (2, 4, 6):
                            yield
                    nc.scalar.copy(av3[:, 16 + 8 * qq:24 + 8 * qq],
                                   acc[:].rearrange(
                                       "p (r w) -> p r w", w=W))
                    yield

            avb = make_avb()
            next(avb, None)
            pending = None                    # deferred cf-chain generator
            drain = None                      # deferred q1 idm stream
            fa_copy = [None]                  # deferred half-0 facc copy
            for half in range(2):
                rh = half * 16

                def emit_prod(k):
                    di, dj = OFFS[k]
                    prod = ppool.tile([128, HF], EDT, tag="pp", bufs=6)
                    p3 = prod.rearrange("p (r w) -> p r w", w=W)
                    nc.vector.tensor_mul(
                        p3[:], av3[:, rh:rh + 16, :],
                        tap(h3, di, dj, rows=16, r0=2 + rh))
                    return prod

                prods = {0: emit_prod(0), 1: emit_prod(1)}
                facc_q0 = psfa.tile([128, QF], F32)
                pks = {}

                def emit_idm_q0(k):
                    pk = pks[k]
                    for q in range(2):
                        c0 = q * 512
                        nc.tensor.matmul(facc_q0[:, c0:c0 + 512], w3[:],
                                         pk[:, c0:c0 + 512],
                                         start=(k == 0), stop=(k == 8))

                def make_drain(pks_h, half_h):
                    """Dense 18-matmul stream accumulating quarter 1 of
                    half_h's F, emitted in chunks as PE filler."""
                    facc_q1 = psnk.tile([128, QF], F32, tag="nk")
                    for k in range(9):
                        pk = pks_h[k]
                        for q in range(2):
                            nc.tensor.matmul(
                                facc_q1[:, q * 512:(q + 1) * 512], w3[:],
                                pk[:, QF + q * 512:QF + (q + 1) * 512],
                                start=(k == 0), stop=(k == 8))
                        if k % 3 == 2:
                            yield
                    nc.scalar.copy(fa[:, half_h * HF + QF:(half_h + 1) * HF],
                                   facc_q1[:])
                    yield


                for k, (di, dj) in enumerate(OFFS):
                    prod = prods.pop(k)
                    nb = npool.tile([128, HF], EDT, tag="nb", bufs=8)
                    for ch in range(2):
                        pst = psnk.tile([128, QF], F32, tag="nk")
                        for q in range(2):
                            c0 = q * 512
                            nc.tensor.matmul(
                                pst[:, c0:c0 + 512], bo[:],
                                prod[:, ch * QF + c0:ch * QF + c0 + 512],
                                start=True, stop=True)
                        nc.scalar.copy(nb[:, ch * QF:(ch + 1) * QF], pst[:])
                    kr = (k - 4) % 9          # put k=4 (center) at rows 0..1
                    hs = slice(half * HF, (half + 1) * HF)
                    nc.sync.dma_start(nst[2 * kr:2 * kr + 2, hs],
                                      nb[0:128:64, :])
                    if kr == 0:
                        nc.sync.dma_start(nst[64:66, hs], nb[0:128:64, :])

                    nb3 = nb.rearrange("p (r w) -> p r w", w=W)
                    pk = ppool.tile([128, HF], EDT, tag="pk", bufs=11)
                    p3 = pk.rearrange("p (r w) -> p r w", w=W)
                    nc.vector.tensor_mul(p3[:], nb3[:],
                                         tap(h3, di, dj, rows=16, r0=2 + rh))
                    pks[k] = pk
                    if k + 2 < 9:
                        prods[k + 2] = emit_prod(k + 2)
                    if k >= 2:
                        emit_idm_q0(k - 2)
                    # NOTE: the cf chain's res q1 reads fa columns the
                    # drain stream produces, and dependencies only track
                    # already-emitted writers — so pending must never be
                    # advanced past yield 5 before drain has finished
                    # (the post-loop code drains `drain` first).
                    if drain is not None and k in (0, 2, 4, 5, 6, 7):
                        next(drain, None)
                    # last half: drain its own q1 stream inside the loop
                    # (psnk ring: 2 rotating bo slots + 1 held accumulator)
                    if half == 1 and k == 5:
                        selfd = make_drain(pks, 1)
                    if half == 1 and k in (6, 7, 8):
                        next(selfd, None)
                    # fill half-1 box parts into half-0's k loop, one op
                    # at a time so pk/prod are not starved behind them
                    if half == 0 and k in (0, 1, 2, 3, 4, 5, 6):
                        next(avb, None)
                    if half == 1 and k == 1 and fa_copy[0] is not None:
                        fa_copy[0]()
                        fa_copy[0] = None
                    if pending is not None and k in (1, 2, 3, 5, 6, 7):
                        next(pending, None)
                if half == 0:
                    emit_idm_q0(7)
                    emit_idm_q0(8)
                    fq0, hh = facc_q0, half
                    fa_copy = [lambda: nc.scalar.copy(
                        fa[:, hh * HF:hh * HF + QF], fq0[:])]
                    if drain is not None:
                        for _ in drain:
                            pass
                    if pending is not None:
                        for _ in pending:
                            pass
                    drain = make_drain(pks, half)
                    pending = cf_steps(half)
                else:
                    # final half: put the chain's first stage at the front
                    # of the engine queues so the reciprocal isn't stuck
                    # behind the leftover accumulation matmuls, and finish
                    # the q1 stream (fa q1 copy) before res q1 is emitted
                    if pending is not None:
                        for _ in pending:
                            pass
                    emit_idm_q0(7)
                    emit_idm_q0(8)
                    p = cf_steps(half)
                    next(p, None)      # q0: nsq/s2/rcp/cfr
                    nc.scalar.copy(fa[:, half * HF:half * HF + QF],
                                   facc_q0[:])
                    for _ in selfd:    # remaining idm MMs + fa q1 copy
                        pass
                    next(p, None)      # q1: nsq/s2/rcp/cfr
                    for _ in p:        # bc2/cfb + res + dma per quarter
                        pass
                    pending = None
            if pending is not None:
                for _ in pending:
                    pass

    nc.compile()
    return nc


_NC_CACHE = [None]


def _get_nc():
    if _NC_CACHE[0] is None:
        _NC_CACHE[0] = _build()
    return _NC_CACHE[0]


def _host_prep(x):
    import ml_dtypes
    B, Cc, H, Ww = x.shape
    in_maps = []
    for core in range(N_CORES):
        b, half = core // 2, core % 2
        r0 = 64 * half
        gidx = np.clip(np.arange(r0 - 2, r0 + 66), 0, H - 1)
        xs = x[b][:, gidx, :]                     # (64, 68, 128)
        packed = np.ascontiguousarray(
            np.concatenate([xs[:, 0:36], xs[:, 32:68]], axis=0))
        in_maps.append({
            "xb": packed.reshape(128, FIN).astype(ml_dtypes.bfloat16),
        })
    return in_maps


def _const_maps(W_head, W_tail):
    import ml_dtypes

    def to_edt(a):
        return a.astype(ml_dtypes.bfloat16)

    w2 = np.zeros((128, 128), np.float32)
    w2[:64, :64] = W_head.T
    w2[64:, 64:] = W_head.T
    w3 = np.zeros((128, 128), np.float32)
    w3[:64, :64] = W_tail.T
    w3[64:, 64:] = W_tail.T
    bo = np.zeros((128, 128), np.float32)
    bo[:64, :64] = 1.0 / 9.0
    bo[64:, 64:] = 1.0 / 9.0
    s4a = np.zeros((18, 128), np.float32)
    s4a[0::2, 0] = 1.0
    s4a[1::2, 1] = 1.0
    s4b = np.zeros((18, 128), np.float32)
    s4b[0::2, 64] = 1.0
    s4b[1::2, 65] = 1.0
    bc2 = np.zeros((2, 128), np.float32)
    bc2[0, :64] = 1.0
    bc2[1, 64:] = 1.0
    return {"w2": to_edt(w2), "w3": to_edt(w3), "bo": to_edt(bo),
            "s4a": to_edt(s4a), "s4b": to_edt(s4b), "bc2": to_edt(bc2),
            "idm": to_edt(np.eye(128, dtype=np.float32))}


def kernel(x, W_head, W_tail):
    x = np.asarray(x, np.float32)
    W_head = np.asarray(W_head, np.float32)
    W_tail = np.asarray(W_tail, np.float32)
    nc = _get_nc()
    consts = _const_maps(W_head, W_tail)
    in_maps = [{**m, **consts} for m in _host_prep(x)]
    res = run_bass_kernel_spmd(nc, in_maps, list(range(N_CORES)))
    out = np.empty_like(x)
    for core in range(N_CORES):
        b, half = core // 2, core % 2
        r0 = 64 * half
        y = np.asarray(res.results[core]["y"]).astype(np.float32)
        y = y.reshape(128, ROUT, W)
        out[b, :, r0:r0 + 32, :] = y[:64]
        out[b, :, r0 + 32:r0 + 64, :] = y[64:]
    return out



# revision 6
# speedup vs baseline: 1.0067x; 1.0067x over previous
"""Trainium2 Bass kernel v2 for nn_LowRankLayer_dilation (B=4, C=64, H=W=128).

Math (exact collapse of the rank-3 NMF update, eps negligible):
    h   = relu(W_head @ x)            (per-pixel channel matmul)
    g   = W_tail @ h                  (per-pixel channel matmul)
    a   = box9(h)                     (3x3 dilation-2 box sum, edge-clamped)
    n_k = sum_c (a/9)_c * h_c(p+d_k)  (9 taps, d in {-2,0,2}^2)
    out = x + (n_4 / sum_j n_j^2) * sum_k n_k * g(p+d_k)

v2 changes vs baseline:
  - bf16 everywhere incl. residual + output (no f32 xr input, y is bf16,
    host upcasts): input DMA 3.4MB -> 1.2MB, output 2.1MB -> 1MB.
  - input streamed in 512-col chunks on the sync engine so head matmuls
    start as soon as the first chunk lands.
  - engine rebalance: scalar owns all PSUM->SBUF copies + relu + squares,
    vector owns 2-input elementwise + reciprocal, gpsimd owns nst/y DMA
    triggers + half the residual chain, sync owns input DMA triggers.
  - box filter split: av for half 0 computed first so the k-loop starts
    ~8us earlier; av for half 1 filled in during half 0's k-loop.
  - nst rows collected with one partition-strided DMA per (k, half).

Sharding: 8 cores = (batch b, H-half). Each core gets a 68-row halo'd
slice packed as 2 channel blocks on 128 partitions: partition p = c +
64*blk, blk A = slice rows 0..35, blk B = rows 32..67. h and g stored
with 2 replicate-padded columns per side (row stride 132) so every
dilated tap is a strided AP view.
"""
import sys
import contextlib
import numpy as np

sys.path.insert(0, '/opt/trn_rl_repo')

import concourse.bass as bass  # noqa: E402,F401
import concourse.bacc as bacc  # noqa: E402
import concourse.tile as tile  # noqa: E402
import concourse.mybir as mybir  # noqa: E402
from concourse.bass_utils import run_bass_kernel_spmd  # noqa: E402

F32 = mybir.dt.float32
BF16 = mybir.dt.bfloat16
AT = mybir.ActivationFunctionType
OP = mybir.AluOpType

N_CORES = 8
RIN = 36          # per-block input rows (with +-2 halo)
ROUT = 32         # per-block output rows
W = 128
WP = W + 4        # padded row stride for h/g
FIN = RIN * W     # 4608
FOUT = ROUT * W   # 4096
OFFS = [(di, dj) for di in (-2, 0, 2) for dj in (-2, 0, 2)]
EDT = BF16


def _build():
    nc = bacc.Bacc("TRN2", target_bir_lowering=False, debug=False,
                   num_devices=N_CORES)
    xb_ext = nc.dram_tensor("xb", [128, FIN], EDT, kind="ExternalInput").ap()
    w2_ext = nc.dram_tensor("w2", [128, 128], EDT, kind="ExternalInput").ap()
    w3_ext = nc.dram_tensor("w3", [128, 128], EDT, kind="ExternalInput").ap()
    bo_ext = nc.dram_tensor("bo", [128, 128], EDT, kind="ExternalInput").ap()
    s4a_ext = nc.dram_tensor("s4a", [18, 128], EDT, kind="ExternalInput").ap()
    s4b_ext = nc.dram_tensor("s4b", [18, 128], EDT, kind="ExternalInput").ap()
    bc2_ext = nc.dram_tensor("bc2", [2, 128], EDT, kind="ExternalInput").ap()
    id_ext = nc.dram_tensor("idm", [128, 128], EDT, kind="ExternalInput").ap()
    y_ext = nc.dram_tensor("y", [128, FOUT], EDT, kind="ExternalOutput").ap()

    with tile.TileContext(nc) as tc, contextlib.ExitStack() as ctx:
        cpool = ctx.enter_context(tc.tile_pool(name="consts", bufs=1))
        big = ctx.enter_context(tc.tile_pool(name="big", bufs=1))
        ppool = ctx.enter_context(tc.tile_pool(name="prod", bufs=4))
        npool = ctx.enter_context(tc.tile_pool(name="nbuf", bufs=4))

        # weights first so the head matmuls can start on chunk 0
        # ascending chunk sizes: the first matmul is gated only on a small
        # 256-col chunk instead of a quarter of the input
        # spray the input triggers across otherwise-idle engines so the
        # descriptor generation runs in parallel and the head's first
        # chunks land sooner
        w2 = cpool.tile([128, 128], EDT)
        nc.sync.dma_start(w2[:], w2_ext[:])
        xbt = big.tile([128, FIN], EDT)
        bounds = [0, 512, 1536, 2560, 3584, FIN]
        engs = [nc.scalar, nc.gpsimd, nc.sync, nc.scalar, nc.gpsimd]
        for c in range(5):
            engs[c].dma_start(xbt[:, bounds[c]:bounds[c + 1]],
                              xb_ext[:, bounds[c]:bounds[c + 1]])
        w3 = cpool.tile([128, 128], EDT)
        nc.scalar.dma_start(w3[:], w3_ext[:])
        bo = cpool.tile([128, 128], EDT)
        nc.sync.dma_start(bo[:], bo_ext[:])
        s4a = cpool.tile([18, 128], EDT)
        nc.gpsimd.dma_start(s4a[:], s4a_ext[:])
        s4b = cpool.tile([18, 128], EDT)
        nc.gpsimd.dma_start(s4b[:], s4b_ext[:])
        bc2 = cpool.tile([2, 128], EDT)
        nc.gpsimd.dma_start(bc2[:], bc2_ext[:])
        idm = cpool.tile([128, 128], EDT)
        nc.gpsimd.dma_start(idm[:], id_ext[:])
        warm = cpool.tile([1, 2], EDT)
        nc.scalar.copy(warm[:], w2[0:1, 0:2])

        # h/g: (RIN, WP) rows; data at cols 2..129, replicate pads at 0,1,
        # 130,131. A (di,dj) tap over out rows = strided view.
        hf = big.tile([128, RIN * WP], EDT)
        h3 = hf.rearrange("p (r w) -> p r w", w=WP)
        xbt3 = xbt.rearrange("p (r w) -> p r w", w=W)

        def tap(t3, di, dj, rows=ROUT, r0=2):
            rr = r0 + di
            return t3[:, rr:rr + rows, 2 + dj:2 + dj + W]

        # ---- head + tail matmuls: h = relu(W_head @ x), g = W_tail @ h ----
        T = big.tile([128, FIN], EDT)
        T3 = T.rearrange("p (r w) -> p r w", w=W)
        av = big.tile([128, FOUT], EDT)
        av3 = av.rearrange("p (r w) -> p r w", w=W)

        def pads(t3, lo, hi):
            for dst, src in ((0, 2), (1, 2), (130, 129), (131, 129)):
                nc.vector.tensor_copy(t3[:, lo:hi, dst:dst + 1],
                                      t3[:, lo:hi, src:src + 1])

        def box_part(lo, hi):
            # T rows lo..hi (column 3-tap sums; T row r needs only h row r)
            nc.vector.tensor_add(T3[:, lo:hi], tap(h3, -2, -2, hi - lo, 2 + lo),
                                 tap(h3, -2, 0, hi - lo, 2 + lo))
            nc.vector.tensor_add(T3[:, lo:hi], T3[:, lo:hi],
                                 tap(h3, -2, 2, hi - lo, 2 + lo))

        # head/tail in 3 row-groups of 12 (1536 cols, 3 PSUM banks each,
        # ring of 2 = 6 banks). Box pieces follow the relus row-wise so
        # the half-0 k loop can start as soon as h rows 0..19 exist.
        with tc.tile_pool(name="psmm", bufs=2, space="PSUM") as psmm:
            G = 1536

            def head_mm(j):
                ps = psmm.tile([128, G], F32)
                for q in range(3):
                    c0 = j * G + q * 512
                    nc.tensor.matmul(ps[:, q * 512:(q + 1) * 512], w2[:],
                                     xbt[:, c0:c0 + 512],
                                     start=True, stop=True)
                return ps

            hps0 = head_mm(0)
            hps1 = head_mm(1)
            nc.vector.tensor_relu(h3[:, 0:12, 2:2 + W],
                                  hps0.rearrange("p (r w) -> p r w", w=W))
            hps2 = head_mm(2)                 # ring slot 0, after relu j0
            nc.scalar.activation(h3[:, 12:24, 2:2 + W],
                                 hps1.rearrange("p (r w) -> p r w", w=W),
                                 AT.Relu)
            nc.scalar.activation(h3[:, 24:36, 2:2 + W],
                                 hps2.rearrange("p (r w) -> p r w", w=W),
                                 AT.Relu)
            for dst, src in ((0, 2), (1, 2), (130, 129), (131, 129)):
                nc.scalar.copy(h3[:, 20:36, dst:dst + 1],
                               h3[:, 20:36, src:src + 1])
            pads(h3, 0, 12)
            box_part(0, 12)
            pads(h3, 12, 20)
            box_part(12, 20)
            nc.vector.tensor_add(av3[:, 0:16], T3[:, 0:16], T3[:, 2:18])
            nc.vector.tensor_add(av3[:, 0:16], av3[:, 0:16], T3[:, 4:20])

        # ---- per-k: n_k (PE reduce+broadcast), F accumulated on the PE ----
        # PSUM budget (8 banks): psnk tag ring 3x[128,1024]f32 = 6 banks
        # (shared by bo outputs, the deferred q1 accumulator, and the cf
        # chain's s2/bc2 outputs), psfa 1x[128,1024] = 2 banks (q0
        # accumulator). F accumulation is split in column quarters: q0
        # accumulates in-loop (lag 2); q1's 18 matmuls run as one dense
        # deferred stream that keeps the PE busy across the half boundary.
        nst = cpool.tile([66, FOUT], EDT)       # n_k rows, row pair by kr
        # rows 64:66 hold a second copy of the center rows so the q1 cf
        # mul can read both operands at base partition 64 (verifier
        # requires equal SB base partitions)
        fa = big.tile([128, FOUT], EDT)         # facc copied out per quarter
        nsq = npool.tile([18, FOUT], EDT, tag="nsq", bufs=1)
        cfr = cpool.tile([2, FOUT], EDT)
        HF = 2048
        QF = 1024

        with tc.tile_pool(name="psnk", bufs=3, space="PSUM") as psnk, \
                tc.tile_pool(name="psfa", bufs=1, space="PSUM") as psfa:

            def cf_steps(half):
                """Deferred tail for one half, pipelined per column
                quarter: nsq square (scalar) -> s2 matmuls -> reciprocal +
                cf mul (vector) -> bc2 broadcast -> cfb copy (scalar) ->
                residual (vector) -> DMA out (sync)."""
                rh = half * 16
                s2ps = psnk.tile([128, QF], F32, tag="nk")
                for qq in range(2):
                    q0 = half * HF + qq * QF
                    if half == 1:
                        # tail-critical: square on vector (it idles here
                        # waiting for this very chain; same-engine rcp
                        # avoids two cross-engine semaphore hops)
                        nc.vector.tensor_mul(nsq[:, q0:q0 + QF],
                                             nst[0:18, q0:q0 + QF],
                                             nst[0:18, q0:q0 + QF])
                    else:
                        nc.scalar.activation(nsq[:, q0:q0 + QF],
                                             nst[0:18, q0:q0 + QF],
                                             AT.Square)
                    sbw = s4a if qq == 0 else s4b
                    for q in range(2):
                        c0 = q0 + q * 512
                        nc.tensor.matmul(s2ps[:, q * 512:(q + 1) * 512],
                                         sbw[:], nsq[:, c0:c0 + 512],
                                         start=(qq == 0), stop=(qq == 1))
                    yield
                rcp = npool.tile([66, QF], F32, tag="rcp", bufs=2)
                nc.vector.reciprocal_approx_fast(rcp[:], s2ps[0:66, :])
                for qq in range(2):
                    q0 = half * HF + qq * QF
                    b = 64 * qq
                    nc.vector.tensor_mul(cfr[:, q0:q0 + QF],
                                         nst[b:b + 2, q0:q0 + QF],
                                         rcp[b:b + 2, :])
                    yield
                for qq in range(2):
                    q0 = half * HF + qq * QF
                    cfbs = npool.tile([128, QF], EDT, tag="cfbs", bufs=2)
                    pst = psnk.tile([128, QF], F32, tag="nk")
                    for q in range(2):
                        c0 = q0 + q * 512
                        nc.tensor.matmul(pst[:, q * 512:(q + 1) * 512],
                                         bc2[:], cfr[:, c0:c0 + 512],
                                         start=True, stop=True)
                    if half == 1:
                        nc.vector.tensor_copy(cfbs[:], pst[:])
                    else:
                        nc.scalar.copy(cfbs[:], pst[:])
                    yield
                    res = npool.tile([128, QF], EDT, tag="res", bufs=2)
                    nc.vector.tensor_mul(res[:], fa[:, q0:q0 + QF], cfbs[:])
                    r3 = res.rearrange("p (r w) -> p r w", w=W)
                    nc.vector.tensor_add(
                        r3[:], r3[:],
                        xbt3[:, 2 + rh + qq * 8:10 + rh + qq * 8, :])
                    nc.sync.dma_start(y_ext[:, q0:q0 + QF], res[:])
                    yield

            # half-1 box filter on the PE: av[16:32] accumulated as 9
            # identity matmuls over the dilated taps. Started at build
            # top so the first burst fills the PE's startup gap (gated
            # only on relu j2 + the scalar pads above).
            def make_avb():
                for qq in range(2):
                    acc = psnk.tile([128, QF], F32, tag="nk")
                    for j, (di2, dj2) in enumerate(OFFS):
                        for q in range(2):
                            tv = tap(h3, di2, dj2, rows=4,
                                     r0=18 + 8 * qq + 4 * q)
                            nc.tensor.matmul(
                                acc[:, q * 512:(q + 1) * 512], idm[:],
                                tv, start=(j == 0), stop=(j == 8))
                        if j in (2, 4, 6):
                            yield
                    nc.scalar.copy(av3[:, 16 + 8 * qq:24 + 8 * qq],
                                   acc[:].rearrange(
                                       "p (r w) -> p r w", w=W))
                    yield

            avb = make_avb()
            next(avb, None)
            pending = None                    # deferred cf-chain generator
            drain = None                      # deferred q1 idm stream
            fa_copy = [None]                  # deferred half-0 facc copy
            for half in range(2):
                rh = half * 16

                def emit_prod(k):
                    di, dj = OFFS[k]
                    prod = ppool.tile([128, HF], EDT, tag="pp", bufs=6)
                    p3 = prod.rearrange("p (r w) -> p r w", w=W)
                    nc.vector.tensor_mul(
                        p3[:], av3[:, rh:rh + 16, :],
                        tap(h3, di, dj, rows=16, r0=2 + rh))
                    return prod

                prods = {0: emit_prod(0), 1: emit_prod(1)}
                facc_q0 = psfa.tile([128, QF], F32)
                pks = {}

                def emit_idm_q0(k):
                    pk = pks[k]
                    for q in range(2):
                        c0 = q * 512
                        nc.tensor.matmul(facc_q0[:, c0:c0 + 512], w3[:],
                                         pk[:, c0:c0 + 512],
                                         start=(k == 0), stop=(k == 8))

                def make_drain(pks_h, half_h):
                    """Dense 18-matmul stream accumulating quarter 1 of
                    half_h's F, emitted in chunks as PE filler."""
                    facc_q1 = psnk.tile([128, QF], F32, tag="nk")
                    for k in range(9):
                        pk = pks_h[k]
                        for q in range(2):
                            nc.tensor.matmul(
                                facc_q1[:, q * 512:(q + 1) * 512], w3[:],
                                pk[:, QF + q * 512:QF + (q + 1) * 512],
                                start=(k == 0), stop=(k == 8))
                        if k % 3 == 2:
                            yield
                    nc.scalar.copy(fa[:, half_h * HF + QF:(half_h + 1) * HF],
                                   facc_q1[:])
                    yield


                for k, (di, dj) in enumerate(OFFS):
                    prod = prods.pop(k)
                    nb = npool.tile([128, HF], EDT, tag="nb", bufs=8)
                    for ch in range(2):
                        pst = psnk.tile([128, QF], F32, tag="nk")
                        for q in range(2):
                            c0 = q * 512
                            nc.tensor.matmul(
                                pst[:, c0:c0 + 512], bo[:],
                                prod[:, ch * QF + c0:ch * QF + c0 + 512],
                                start=True, stop=True)
                        nc.scalar.copy(nb[:, ch * QF:(ch + 1) * QF], pst[:])
                    kr = (k - 4) % 9          # put k=4 (center) at rows 0..1
                    hs = slice(half * HF, (half + 1) * HF)
                    nc.sync.dma_start(nst[2 * kr:2 * kr + 2, hs],
                                      nb[0:128:64, :])
                    if kr == 0:
                        nc.sync.dma_start(nst[64:66, hs], nb[0:128:64, :])

                    nb3 = nb.rearrange("p (r w) -> p r w", w=W)
                    pk = ppool.tile([128, HF], EDT, tag="pk", bufs=11)
                    p3 = pk.rearrange("p (r w) -> p r w", w=W)
                    nc.vector.tensor_mul(p3[:], nb3[:],
                                         tap(h3, di, dj, rows=16, r0=2 + rh))
                    pks[k] = pk
                    if k + 2 < 9:
                        prods[k + 2] = emit_prod(k + 2)
                    if k >= 2:
                        emit_idm_q0(k - 2)
                    # NOTE: the cf chain's res q1 reads fa columns the
                    # drain stream produces, and dependencies only track
                    # already-emitted writers — so pending must never be
                    # advanced past yield 5 before drain has finished
                    # (the post-loop code drains `drain` first).
                    if drain is not None and k in (0, 2, 4, 5, 6, 7):
                        next(drain, None)
                    # last half: drain its own q1 stream inside the loop
                    # (psnk ring: 2 rotating bo slots + 1 held accumulator)
                    if half == 1 and k == 5:
                        selfd = make_drain(pks, 1)
                    if half == 1 and k in (6, 7, 8):
                        next(selfd, None)
                    # fill half-1 box parts into half-0's k loop, one op
                    # at a time so pk/prod are not starved behind them
                    if half == 0 and k in (0, 1, 2, 3, 4, 5, 6):
                        next(avb, None)
                    if half == 1 and k == 1 and fa_copy[0] is not None:
                        fa_copy[0]()
                        fa_copy[0] = None
                    if pending is not None and k in (1, 2, 3, 5, 6, 7):
                        next(pending, None)
                if half == 0:
                    emit_idm_q0(7)
                    emit_idm_q0(8)
                    fq0, hh = facc_q0, half
                    fa_copy = [lambda: nc.scalar.copy(
                        fa[:, hh * HF:hh * HF + QF], fq0[:])]
                    if drain is not None:
                        for _ in drain:
                            pass
                    if pending is not None:
                        for _ in pending:
                            pass
                    drain = make_drain(pks, half)
                    pending = cf_steps(half)
                else:
                    # final half: put the chain's first stage at the front
                    # of the engine queues so the reciprocal isn't stuck
                    # behind the leftover accumulation matmuls, and finish
                    # the q1 stream (fa q1 copy) before res q1 is emitted
                    if pending is not None:
                        for _ in pending:
                            pass
                    emit_idm_q0(7)
                    emit_idm_q0(8)
                    p = cf_steps(half)
                    next(p, None)      # q0: nsq/s2/rcp/cfr
                    nc.scalar.copy(fa[:, half * HF:half * HF + QF],
                                   facc_q0[:])
                    for _ in selfd:    # remaining idm MMs + fa q1 copy
                        pass
                    next(p, None)      # q1: nsq/s2/rcp/cfr
                    for _ in p:        # bc2/cfb + res + dma per quarter
                        pass
                    pending = None
            if pending is not None:
                for _ in pending:
                    pass

    nc.compile()
    return nc


_NC_CACHE = [None]


def _get_nc():
    if _NC_CACHE[0] is None:
        _NC_CACHE[0] = _build()
    return _NC_CACHE[0]


def _host_prep(x):
    import ml_dtypes
    B, Cc, H, Ww = x.shape
    in_maps = []
    for core in range(N_CORES):
        b, half = core // 2, core % 2
        r0 = 64 * half
        gidx = np.clip(np.arange(r0 - 2, r0 + 66), 0, H - 1)
        xs = x[b][:, gidx, :]                     # (64, 68, 128)
        packed = np.ascontiguousarray(
            np.concatenate([xs[:, 0:36], xs[:, 32:68]], axis=0))
        in_maps.append({
            "xb": packed.reshape(128, FIN).astype(ml_dtypes.bfloat16),
        })
    return in_maps


def _const_maps(W_head, W_tail):
    import ml_dtypes

    def to_edt(a):
        return a.astype(ml_dtypes.bfloat16)

    w2 = np.zeros((128, 128), np.float32)
    w2[:64, :64] = W_head.T
    w2[64:, 64:] = W_head.T
    w3 = np.zeros((128, 128), np.float32)
    w3[:64, :64] = W_tail.T
    w3[64:, 64:] = W_tail.T
    bo = np.zeros((128, 128), np.float32)
    bo[:64, :64] = 1.0 / 9.0
    bo[64:, 64:] = 1.0 / 9.0
    s4a = np.zeros((18, 128), np.float32)
    s4a[0::2, 0] = 1.0
    s4a[1::2, 1] = 1.0
    s4b = np.zeros((18, 128), np.float32)
    s4b[0::2, 64] = 1.0
    s4b[1::2, 65] = 1.0
    bc2 = np.zeros((2, 128), np.float32)
    bc2[0, :64] = 1.0
    bc2[1, 64:] = 1.0
    return {"w2": to_edt(w2), "w3": to_edt(w3), "bo": to_edt(bo),
            "s4a": to_edt(s4a), "s4b": to_edt(s4b), "bc2": to_edt(bc2),
            "idm": to_edt(np.eye(128, dtype=np.float32))}


def kernel(x, W_head, W_tail):
    x = np.asarray(x, np.float32)
    W_head = np.asarray(W_head, np.float32)
    W_tail = np.asarray(W_tail, np.float32)
    nc = _get_nc()
    consts = _const_maps(W_head, W_tail)
    in_maps = [{**m, **consts} for m in _host_prep(x)]
    res = run_bass_kernel_spmd(nc, in_maps, list(range(N_CORES)))
    out = np.empty_like(x)
    for core in range(N_CORES):
        b, half = core // 2, core % 2
        r0 = 64 * half
        y = np.asarray(res.results[core]["y"]).astype(np.float32)
        y = y.reshape(128, ROUT, W)
        out[b, :, r0:r0 + 32, :] = y[:64]
        out[b, :, r0 + 32:r0 + 64, :] = y[64:]
    return out



# revision 7
# speedup vs baseline: 1.0109x; 1.0042x over previous
"""Trainium2 Bass kernel v2 for nn_LowRankLayer_dilation (B=4, C=64, H=W=128).

Math (exact collapse of the rank-3 NMF update, eps negligible):
    h   = relu(W_head @ x)            (per-pixel channel matmul)
    g   = W_tail @ h                  (per-pixel channel matmul)
    a   = box9(h)                     (3x3 dilation-2 box sum, edge-clamped)
    n_k = sum_c (a/9)_c * h_c(p+d_k)  (9 taps, d in {-2,0,2}^2)
    out = x + (n_4 / sum_j n_j^2) * sum_k n_k * g(p+d_k)

v2 changes vs baseline:
  - bf16 everywhere incl. residual + output (no f32 xr input, y is bf16,
    host upcasts): input DMA 3.4MB -> 1.2MB, output 2.1MB -> 1MB.
  - input streamed in 512-col chunks on the sync engine so head matmuls
    start as soon as the first chunk lands.
  - engine rebalance: scalar owns all PSUM->SBUF copies + relu + squares,
    vector owns 2-input elementwise + reciprocal, gpsimd owns nst/y DMA
    triggers + half the residual chain, sync owns input DMA triggers.
  - box filter split: av for half 0 computed first so the k-loop starts
    ~8us earlier; av for half 1 filled in during half 0's k-loop.
  - nst rows collected with one partition-strided DMA per (k, half).

Sharding: 8 cores = (batch b, H-half). Each core gets a 68-row halo'd
slice packed as 2 channel blocks on 128 partitions: partition p = c +
64*blk, blk A = slice rows 0..35, blk B = rows 32..67. h and g stored
with 2 replicate-padded columns per side (row stride 132) so every
dilated tap is a strided AP view.
"""
import sys
import contextlib
import numpy as np

sys.path.insert(0, '/opt/trn_rl_repo')

import concourse.bass as bass  # noqa: E402,F401
import concourse.bacc as bacc  # noqa: E402
import concourse.tile as tile  # noqa: E402
import concourse.mybir as mybir  # noqa: E402
from concourse.bass_utils import run_bass_kernel_spmd  # noqa: E402

F32 = mybir.dt.float32
BF16 = mybir.dt.bfloat16
AT = mybir.ActivationFunctionType
OP = mybir.AluOpType

N_CORES = 8
RIN = 36          # per-block input rows (with +-2 halo)
ROUT = 32         # per-block output rows
W = 128
WP = W + 4        # padded row stride for h/g
FIN = RIN * W     # 4608
FOUT = ROUT * W   # 4096
OFFS = [(di, dj) for di in (-2, 0, 2) for dj in (-2, 0, 2)]
EDT = BF16


def _build():
    nc = bacc.Bacc("TRN2", target_bir_lowering=False, debug=False,
                   num_devices=N_CORES)
    xb_ext = nc.dram_tensor("xb", [128, FIN], EDT, kind="ExternalInput").ap()
    w2_ext = nc.dram_tensor("w2", [128, 128], EDT, kind="ExternalInput").ap()
    w3_ext = nc.dram_tensor("w3", [128, 128], EDT, kind="ExternalInput").ap()
    bo_ext = nc.dram_tensor("bo", [128, 128], EDT, kind="ExternalInput").ap()
    s4a_ext = nc.dram_tensor("s4a", [18, 128], EDT, kind="ExternalInput").ap()
    s4b_ext = nc.dram_tensor("s4b", [18, 128], EDT, kind="ExternalInput").ap()
    bc2_ext = nc.dram_tensor("bc2", [2, 128], EDT, kind="ExternalInput").ap()
    id_ext = nc.dram_tensor("idm", [128, 128], EDT, kind="ExternalInput").ap()
    y_ext = nc.dram_tensor("y", [128, FOUT], EDT, kind="ExternalOutput").ap()

    with tile.TileContext(nc) as tc, contextlib.ExitStack() as ctx:
        cpool = ctx.enter_context(tc.tile_pool(name="consts", bufs=1))
        big = ctx.enter_context(tc.tile_pool(name="big", bufs=1))
        ppool = ctx.enter_context(tc.tile_pool(name="prod", bufs=4))
        npool = ctx.enter_context(tc.tile_pool(name="nbuf", bufs=4))

        # weights first so the head matmuls can start on chunk 0
        # ascending chunk sizes: the first matmul is gated only on a small
        # 256-col chunk instead of a quarter of the input
        # spray the input triggers across otherwise-idle engines so the
        # descriptor generation runs in parallel and the head's first
        # chunks land sooner
        w2 = cpool.tile([128, 128], EDT)
        nc.sync.dma_start(w2[:], w2_ext[:])
        xbt = big.tile([128, FIN], EDT)
        bounds = [0, 512, 1536, 2560, 3584, FIN]
        engs = [nc.scalar, nc.gpsimd, nc.sync, nc.scalar, nc.gpsimd]
        for c in range(5):
            engs[c].dma_start(xbt[:, bounds[c]:bounds[c + 1]],
                              xb_ext[:, bounds[c]:bounds[c + 1]])
        w3 = cpool.tile([128, 128], EDT)
        nc.scalar.dma_start(w3[:], w3_ext[:])
        bo = cpool.tile([128, 128], EDT)
        nc.sync.dma_start(bo[:], bo_ext[:])
        s4a = cpool.tile([18, 128], EDT)
        nc.gpsimd.dma_start(s4a[:], s4a_ext[:])
        s4b = cpool.tile([18, 128], EDT)
        nc.gpsimd.dma_start(s4b[:], s4b_ext[:])
        bc2 = cpool.tile([2, 128], EDT)
        nc.gpsimd.dma_start(bc2[:], bc2_ext[:])
        idm = cpool.tile([128, 128], EDT)
        nc.gpsimd.dma_start(idm[:], id_ext[:])
        warm = cpool.tile([1, 2], EDT)
        nc.scalar.copy(warm[:], w2[0:1, 0:2])

        # h/g: (RIN, WP) rows; data at cols 2..129, replicate pads at 0,1,
        # 130,131. A (di,dj) tap over out rows = strided view.
        hf = big.tile([128, RIN * WP], EDT)
        h3 = hf.rearrange("p (r w) -> p r w", w=WP)
        xbt3 = xbt.rearrange("p (r w) -> p r w", w=W)

        def tap(t3, di, dj, rows=ROUT, r0=2):
            rr = r0 + di
            return t3[:, rr:rr + rows, 2 + dj:2 + dj + W]

        # ---- head + tail matmuls: h = relu(W_head @ x), g = W_tail @ h ----
        T = big.tile([128, FIN], EDT)
        T3 = T.rearrange("p (r w) -> p r w", w=W)
        av = big.tile([128, FOUT], EDT)
        av3 = av.rearrange("p (r w) -> p r w", w=W)

        def pads(t3, lo, hi):
            for dst, src in ((0, 2), (1, 2), (130, 129), (131, 129)):
                nc.vector.tensor_copy(t3[:, lo:hi, dst:dst + 1],
                                      t3[:, lo:hi, src:src + 1])

        def box_part(lo, hi):
            # T rows lo..hi (column 3-tap sums; T row r needs only h row r)
            nc.vector.tensor_add(T3[:, lo:hi], tap(h3, -2, -2, hi - lo, 2 + lo),
                                 tap(h3, -2, 0, hi - lo, 2 + lo))
            nc.vector.tensor_add(T3[:, lo:hi], T3[:, lo:hi],
                                 tap(h3, -2, 2, hi - lo, 2 + lo))

        # head/tail in 3 row-groups of 12 (1536 cols, 3 PSUM banks each,
        # ring of 2 = 6 banks). Box pieces follow the relus row-wise so
        # the half-0 k loop can start as soon as h rows 0..19 exist.
        with tc.tile_pool(name="psmm", bufs=2, space="PSUM") as psmm:
            G = 1536

            def head_mm(j):
                ps = psmm.tile([128, G], F32)
                for q in range(3):
                    c0 = j * G + q * 512
                    nc.tensor.matmul(ps[:, q * 512:(q + 1) * 512], w2[:],
                                     xbt[:, c0:c0 + 512],
                                     start=True, stop=True)
                return ps

            hps0 = head_mm(0)
            hps1 = head_mm(1)
            nc.scalar.activation(
                h3[:, 0:6, 2:2 + W],
                hps0[:, 0:768].rearrange("p (r w) -> p r w", w=W), AT.Relu)
            nc.vector.tensor_relu(
                h3[:, 6:12, 2:2 + W],
                hps0[:, 768:1536].rearrange("p (r w) -> p r w", w=W))
            hps2 = head_mm(2)                 # ring slot 0, after relu j0
            nc.scalar.activation(h3[:, 12:24, 2:2 + W],
                                 hps1.rearrange("p (r w) -> p r w", w=W),
                                 AT.Relu)
            nc.scalar.activation(h3[:, 24:36, 2:2 + W],
                                 hps2.rearrange("p (r w) -> p r w", w=W),
                                 AT.Relu)
            for dst, src in ((0, 2), (1, 2), (130, 129), (131, 129)):
                nc.scalar.copy(h3[:, 20:36, dst:dst + 1],
                               h3[:, 20:36, src:src + 1])
            pads(h3, 0, 12)
            box_part(0, 12)
            pads(h3, 12, 20)
            box_part(12, 20)
            nc.vector.tensor_add(av3[:, 0:16], T3[:, 0:16], T3[:, 2:18])
            nc.vector.tensor_add(av3[:, 0:16], av3[:, 0:16], T3[:, 4:20])

        # ---- per-k: n_k (PE reduce+broadcast), F accumulated on the PE ----
        # PSUM budget (8 banks): psnk tag ring 3x[128,1024]f32 = 6 banks
        # (shared by bo outputs, the deferred q1 accumulator, and the cf
        # chain's s2/bc2 outputs), psfa 1x[128,1024] = 2 banks (q0
        # accumulator). F accumulation is split in column quarters: q0
        # accumulates in-loop (lag 2); q1's 18 matmuls run as one dense
        # deferred stream that keeps the PE busy across the half boundary.
        nst = cpool.tile([66, FOUT], EDT)       # n_k rows, row pair by kr
        # rows 64:66 hold a second copy of the center rows so the q1 cf
        # mul can read both operands at base partition 64 (verifier
        # requires equal SB base partitions)
        fa = big.tile([128, FOUT], EDT)         # facc copied out per quarter
        nsq = npool.tile([18, FOUT], EDT, tag="nsq", bufs=1)
        cfr = cpool.tile([2, FOUT], EDT)
        HF = 2048
        QF = 1024

        with tc.tile_pool(name="psnk", bufs=3, space="PSUM") as psnk, \
                tc.tile_pool(name="psfa", bufs=1, space="PSUM") as psfa:

            def cf_steps(half):
                """Deferred tail for one half, pipelined per column
                quarter: nsq square (scalar) -> s2 matmuls -> reciprocal +
                cf mul (vector) -> bc2 broadcast -> cfb copy (scalar) ->
                residual (vector) -> DMA out (sync)."""
                rh = half * 16
                s2ps = psnk.tile([128, QF], F32, tag="nk")
                for qq in range(2):
                    q0 = half * HF + qq * QF
                    if half == 1:
                        # tail-critical: square on vector (it idles here
                        # waiting for this very chain; same-engine rcp
                        # avoids two cross-engine semaphore hops)
                        nc.vector.tensor_mul(nsq[:, q0:q0 + QF],
                                             nst[0:18, q0:q0 + QF],
                                             nst[0:18, q0:q0 + QF])
                    else:
                        nc.scalar.activation(nsq[:, q0:q0 + QF],
                                             nst[0:18, q0:q0 + QF],
                                             AT.Square)
                    sbw = s4a if qq == 0 else s4b
                    for q in range(2):
                        c0 = q0 + q * 512
                        nc.tensor.matmul(s2ps[:, q * 512:(q + 1) * 512],
                                         sbw[:], nsq[:, c0:c0 + 512],
                                         start=(qq == 0), stop=(qq == 1))
                    yield
                rcp = npool.tile([66, QF], F32, tag="rcp", bufs=2)
                nc.vector.reciprocal_approx_fast(rcp[:], s2ps[0:66, :])
                for qq in range(2):
                    q0 = half * HF + qq * QF
                    b = 64 * qq
                    nc.vector.tensor_mul(cfr[:, q0:q0 + QF],
                                         nst[b:b + 2, q0:q0 + QF],
                                         rcp[b:b + 2, :])
                    yield
                for qq in range(2):
                    q0 = half * HF + qq * QF
                    cfbs = npool.tile([128, QF], EDT, tag="cfbs", bufs=2)
                    pst = psnk.tile([128, QF], F32, tag="nk")
                    for q in range(2):
                        c0 = q0 + q * 512
                        nc.tensor.matmul(pst[:, q * 512:(q + 1) * 512],
                                         bc2[:], cfr[:, c0:c0 + 512],
                                         start=True, stop=True)
                    if half == 1:
                        nc.vector.tensor_copy(cfbs[:], pst[:])
                    else:
                        nc.scalar.copy(cfbs[:], pst[:])
                    yield
                    res = npool.tile([128, QF], EDT, tag="res", bufs=2)
                    nc.vector.tensor_mul(res[:], fa[:, q0:q0 + QF], cfbs[:])
                    r3 = res.rearrange("p (r w) -> p r w", w=W)
                    nc.vector.tensor_add(
                        r3[:], r3[:],
                        xbt3[:, 2 + rh + qq * 8:10 + rh + qq * 8, :])
                    nc.sync.dma_start(y_ext[:, q0:q0 + QF], res[:])
                    yield

            # half-1 box filter on the PE: av[16:32] accumulated as 9
            # identity matmuls over the dilated taps. Started at build
            # top so the first burst fills the PE's startup gap (gated
            # only on relu j2 + the scalar pads above).
            def make_avb():
                for qq in range(2):
                    acc = psnk.tile([128, QF], F32, tag="nk")
                    for j, (di2, dj2) in enumerate(OFFS):
                        for q in range(2):
                            tv = tap(h3, di2, dj2, rows=4,
                                     r0=18 + 8 * qq + 4 * q)
                            nc.tensor.matmul(
                                acc[:, q * 512:(q + 1) * 512], idm[:],
                                tv, start=(j == 0), stop=(j == 8))
                        if j in (2, 4, 6):
                            yield
                    nc.scalar.copy(av3[:, 16 + 8 * qq:24 + 8 * qq],
                                   acc[:].rearrange(
                                       "p (r w) -> p r w", w=W))
                    yield

            avb = make_avb()
            next(avb, None)
            pending = None                    # deferred cf-chain generator
            drain = None                      # deferred q1 idm stream
            fa_copy = [None]                  # deferred half-0 facc copy
            for half in range(2):
                rh = half * 16

                def emit_prod(k):
                    di, dj = OFFS[k]
                    prod = ppool.tile([128, HF], EDT, tag="pp", bufs=6)
                    p3 = prod.rearrange("p (r w) -> p r w", w=W)
                    nc.vector.tensor_mul(
                        p3[:], av3[:, rh:rh + 16, :],
                        tap(h3, di, dj, rows=16, r0=2 + rh))
                    return prod

                prods = {0: emit_prod(0), 1: emit_prod(1)}
                facc_q0 = psfa.tile([128, QF], F32)
                pks = {}

                def emit_idm_q0(k):
                    pk = pks[k]
                    for q in range(2):
                        c0 = q * 512
                        nc.tensor.matmul(facc_q0[:, c0:c0 + 512], w3[:],
                                         pk[:, c0:c0 + 512],
                                         start=(k == 0), stop=(k == 8))

                def make_drain(pks_h, half_h):
                    """Dense 18-matmul stream accumulating quarter 1 of
                    half_h's F, emitted in chunks as PE filler."""
                    facc_q1 = psnk.tile([128, QF], F32, tag="nk")
                    for k in range(9):
                        pk = pks_h[k]
                        for q in range(2):
                            nc.tensor.matmul(
                                facc_q1[:, q * 512:(q + 1) * 512], w3[:],
                                pk[:, QF + q * 512:QF + (q + 1) * 512],
                                start=(k == 0), stop=(k == 8))
                        if k % 3 == 2:
                            yield
                    nc.scalar.copy(fa[:, half_h * HF + QF:(half_h + 1) * HF],
                                   facc_q1[:])
                    yield


                for k, (di, dj) in enumerate(OFFS):
                    prod = prods.pop(k)
                    nb = npool.tile([128, HF], EDT, tag="nb", bufs=8)
                    for ch in range(2):
                        pst = psnk.tile([128, QF], F32, tag="nk")
                        for q in range(2):
                            c0 = q * 512
                            nc.tensor.matmul(
                                pst[:, c0:c0 + 512], bo[:],
                                prod[:, ch * QF + c0:ch * QF + c0 + 512],
                                start=True, stop=True)
                        nc.scalar.copy(nb[:, ch * QF:(ch + 1) * QF], pst[:])
                    kr = (k - 4) % 9          # put k=4 (center) at rows 0..1
                    hs = slice(half * HF, (half + 1) * HF)
                    nc.sync.dma_start(nst[2 * kr:2 * kr + 2, hs],
                                      nb[0:128:64, :])
                    if kr == 0:
                        nc.sync.dma_start(nst[64:66, hs], nb[0:128:64, :])

                    nb3 = nb.rearrange("p (r w) -> p r w", w=W)
                    pk = ppool.tile([128, HF], EDT, tag="pk", bufs=11)
                    p3 = pk.rearrange("p (r w) -> p r w", w=W)
                    nc.vector.tensor_mul(p3[:], nb3[:],
                                         tap(h3, di, dj, rows=16, r0=2 + rh))
                    pks[k] = pk
                    if k + 2 < 9:
                        prods[k + 2] = emit_prod(k + 2)
                    if k >= 2:
                        emit_idm_q0(k - 2)
                    # NOTE: the cf chain's res q1 reads fa columns the
                    # drain stream produces, and dependencies only track
                    # already-emitted writers — so pending must never be
                    # advanced past yield 5 before drain has finished
                    # (the post-loop code drains `drain` first).
                    if drain is not None and k in (0, 2, 4, 5, 6, 7):
                        next(drain, None)
                    # last half: drain its own q1 stream inside the loop
                    # (psnk ring: 2 rotating bo slots + 1 held accumulator)
                    if half == 1 and k == 5:
                        selfd = make_drain(pks, 1)
                    if half == 1 and k in (6, 7, 8):
                        next(selfd, None)
                    # fill half-1 box parts into half-0's k loop, one op
                    # at a time so pk/prod are not starved behind them
                    if half == 0 and k in (0, 1, 2, 3, 4, 5, 6):
                        next(avb, None)
                    if half == 1 and k == 1 and fa_copy[0] is not None:
                        fa_copy[0]()
                        fa_copy[0] = None
                    if pending is not None and k in (1, 2, 3, 5, 6, 7):
                        next(pending, None)
                if half == 0:
                    emit_idm_q0(7)
                    emit_idm_q0(8)
                    fq0, hh = facc_q0, half
                    fa_copy = [lambda: nc.scalar.copy(
                        fa[:, hh * HF:hh * HF + QF], fq0[:])]
                    if drain is not None:
                        for _ in drain:
                            pass
                    if pending is not None:
                        for _ in pending:
                            pass
                    drain = make_drain(pks, half)
                    pending = cf_steps(half)
                else:
                    # final half: put the chain's first stage at the front
                    # of the engine queues so the reciprocal isn't stuck
                    # behind the leftover accumulation matmuls, and finish
                    # the q1 stream (fa q1 copy) before res q1 is emitted
                    if pending is not None:
                        for _ in pending:
                            pass
                    emit_idm_q0(7)
                    emit_idm_q0(8)
                    p = cf_steps(half)
                    next(p, None)      # q0: nsq/s2/rcp/cfr
                    nc.scalar.copy(fa[:, half * HF:half * HF + QF],
                                   facc_q0[:])
                    for _ in selfd:    # remaining idm MMs + fa q1 copy
                        pass
                    next(p, None)      # q1: nsq/s2/rcp/cfr
                    for _ in p:        # bc2/cfb + res + dma per quarter
                        pass
                    pending = None
            if pending is not None:
                for _ in pending:
                    pass

    nc.compile()
    return nc


_NC_CACHE = [None]


def _get_nc():
    if _NC_CACHE[0] is None:
        _NC_CACHE[0] = _build()
    return _NC_CACHE[0]


def _host_prep(x):
    import ml_dtypes
    B, Cc, H, Ww = x.shape
    in_maps = []
    for core in range(N_CORES):
        b, half = core // 2, core % 2
        r0 = 64 * half
        gidx = np.clip(np.arange(r0 - 2, r0 + 66), 0, H - 1)
        xs = x[b][:, gidx, :]                     # (64, 68, 128)
        packed = np.ascontiguousarray(
            np.concatenate([xs[:, 0:36], xs[:, 32:68]], axis=0))
        in_maps.append({
            "xb": packed.reshape(128, FIN).astype(ml_dtypes.bfloat16),
        })
    return in_maps


def _const_maps(W_head, W_tail):
    import ml_dtypes

    def to_edt(a):
        return a.astype(ml_dtypes.bfloat16)

    w2 = np.zeros((128, 128), np.float32)
    w2[:64, :64] = W_head.T
    w2[64:, 64:] = W_head.T
    w3 = np.zeros((128, 128), np.float32)
    w3[:64, :64] = W_tail.T
    w3[64:, 64:] = W_tail.T
    bo = np.zeros((128, 128), np.float32)
    bo[:64, :64] = 1.0 / 9.0
    bo[64:, 64:] = 1.0 / 9.0
    s4a = np.zeros((18, 128), np.float32)
    s4a[0::2, 0] = 1.0
    s4a[1::2, 1] = 1.0
    s4b = np.zeros((18, 128), np.float32)
    s4b[0::2, 64] = 1.0
    s4b[1::2, 65] = 1.0
    bc2 = np.zeros((2, 128), np.float32)
    bc2[0, :64] = 1.0
    bc2[1, 64:] = 1.0
    return {"w2": to_edt(w2), "w3": to_edt(w3), "bo": to_edt(bo),
            "s4a": to_edt(s4a), "s4b": to_edt(s4b), "bc2": to_edt(bc2),
            "idm": to_edt(np.eye(128, dtype=np.float32))}


def kernel(x, W_head, W_tail):
    x = np.asarray(x, np.float32)
    W_head = np.asarray(W_head, np.float32)
    W_tail = np.asarray(W_tail, np.float32)
    nc = _get_nc()
    consts = _const_maps(W_head, W_tail)
    in_maps = [{**m, **consts} for m in _host_prep(x)]
    res = run_bass_kernel_spmd(nc, in_maps, list(range(N_CORES)))
    out = np.empty_like(x)
    for core in range(N_CORES):
        b, half = core // 2, core % 2
        r0 = 64 * half
        y = np.asarray(res.results[core]["y"]).astype(np.float32)
        y = y.reshape(128, ROUT, W)
        out[b, :, r0:r0 + 32, :] = y[:64]
        out[b, :, r0 + 32:r0 + 64, :] = y[64:]
    return out

